# revision 38
# baseline (speedup 1.0000x reference)
"""Butterfly network forward pass on 8 Trainium2 NeuronCores.

Strategy: the 10 untied butterfly stages factor as B = S9 . G where
G = stages 0-8 is block-diagonal over two dense 512x512 blocks (stage
strides 1..256 never cross the 512 boundary) and S9 (stride 512) is a
2x2 rotation per position pairing features p and p^512.

Per core (batch shard 2048, features on SBUF partitions):
  - PE: for each 128-wide output tile t, accumulate 4 bf16 matmuls
    (contraction over its 512-block) into a fp32 PSUM tile.  This is
    32 matmuls per 512-batch chunk -- exactly half the dense-GEMM PE
    work, with no inter-stage PSUM->SBUF round trips.
  - Stage 9 + bias run on the Scalar/Vector engines straight out of
    PSUM, fused into 2 passes per tile using the pair structure
    (t, t^4):  u_t = Act(z_t * d9a_t + bias_t)  [ScalarE], then
    out_t = (z_{t^4} * d9b_t) + u_t  [VectorE scalar_tensor_tensor].
  - All activations/weights move as bf16 (rel-err budget 2e-2; the
    measured pipeline error is ~4e-3), halving HBM traffic.

Engine budgets per core: PE 27.3us, ACT ~20us, DVE ~24us, DMA ~25us.
"""

import math

import numpy as np
import ml_dtypes

import concourse.bacc as bacc
import concourse.mybir as mybir
import concourse.tile as tile
from concourse.bass_utils import run_bass_kernel_spmd

N_CORES = 8
BATCH = 16384
N = 1024
M_STAGES = 10
SHARD = BATCH // N_CORES   # 2048 rows per core
P = 128                    # SBUF partitions
NB = 512                   # moving-dim (batch) chunk per matmul (fp32 PSUM)
NCH = SHARD // NB          # 4 batch chunks per core
NT = N // P                # 8 feature tiles
# tile processing order: stage-9 pairs (t, t^4) interleaved
ORDER = [0, 4, 1, 5, 2, 6, 3, 7]

F32 = mybir.dt.float32
BF16 = mybir.dt.bfloat16
IDENT = mybir.ActivationFunctionType.Identity
MULT = mybir.AluOpType.mult
ADD = mybir.AluOpType.add

_NC_CACHE = None


def build_nc(reps_outer: int = 1, reps_inner: int = 1, ew: str = "full"):
    """Build the per-core kernel.

    reps_outer>1 wraps the FULL body (input DMA + compute + output DMA)
    in a hardware For_i loop so a bench harness can measure steady-state
    per-iteration HW time by subtraction; the graded path uses (1, 1).
    reps_inner>1 unrolls inner copies with double-buffered I/O tiles so
    consecutive iterations overlap across the For_i boundary.
    ew: "full" (real kernel) | "act" (skip DVE stage) | "none" (skip
    ACT+DVE; output DMA sources the x tile) -- timing diagnostics only.
    """
    nc = bacc.Bacc("TRN2", target_bir_lowering=False, debug=False,
                   num_devices=N_CORES)
    xT = nc.declare_dram_parameter("xT", [N, SHARD], BF16, isOutput=False)
    # weights packed host-side as [p][q][j][c]: q indexes ORDER, j the 4
    # contraction tiles of that output tile's 512-block, c the 128 output
    # features -> one fully-contiguous 1 MB DMA.
    wB = nc.declare_dram_parameter("wB", [P, NT * 4 * P], BF16,
                                   isOutput=False)
    # per-partition scalars: cols [0:8] d9a, [8:16] d9b, [16:24] bias
    # (indexed by global tile t)
    sc = nc.declare_dram_parameter("sc", [P, 24], F32, isOutput=False)
    outT = nc.declare_dram_parameter("outT", [N, SHARD], BF16, isOutput=True)

    xsrc = xT.rearrange("(k p) (nb b) -> nb p k b", p=P, b=NB)
    # SBUF out slices ordered by global tile index T = 4*i + tp so the
    # DRAM side collapses to one uniform-stride dim (AP balancer limit)
    odst = outT.rearrange("(t p) (nb b) -> nb p t b", p=P, b=NB)

    nbuf = min(reps_inner, 2)
    with tile.TileContext(nc) as tc:
        with (
            tc.tile_pool(name="wp", bufs=nbuf) as wp,
            tc.tile_pool(name="xp", bufs=nbuf) as xp,
            tc.tile_pool(name="scp", bufs=nbuf) as scp,
            tc.tile_pool(name="up", bufs=12) as up,
            tc.tile_pool(name="op", bufs=4) as op,
            tc.tile_pool(name="pp", bufs=6, space="PSUM") as pp,
            tc.tile_pool(name="ppw", bufs=1, space="PSUM") as ppw,
        ):
            def load_io():
                # weights for the first pair (q=0,1) land first so the PE
                # can start early; x chunk 0 as two half-loads on disjoint
                # DMA queue sets.
                sct = scp.tile([P, 24], F32, tag="sct")
                nc.sync.dma_start(out=sct[:], in_=sc[:])
                wt = wp.tile([P, NT * 4 * P], BF16, tag="wt")
                xtiles = [
                    xp.tile([P, NT * NB], BF16, tag=f"xc{n}", name=f"xc{n}")
                    for n in range(NCH)
                ]
                nc.sync.dma_start(out=wt[:, 0:8 * P], in_=wB[:, 0:8 * P])
                h = NT // 2
                x0 = xtiles[0].rearrange("p (k b) -> p k b", b=NB)
                nc.sync.dma_start(out=x0[:, 0:h], in_=xsrc[0, :, 0:h])
                nc.sync.dma_start(out=x0[:, h:NT], in_=xsrc[0, :, h:NT])
                nc.sync.dma_start(out=wt[:, 8 * P:], in_=wB[:, 8 * P:])
                for n in range(1, NCH):
                    dst = xtiles[n].rearrange("p (k b) -> p k b", b=NB)
                    nc.sync.dma_start(out=dst[:], in_=xsrc[n])
                return sct, wt, xtiles

            def warmup(sct):
                # Warm the PE (HAM clock gate) with throwaway tiny matmuls
                # while the prologue DMA streams in.
                wps = ppw.tile([8, 8], F32, tag="warm")
                for _ in range(32):
                    nc.tensor.matmul(wps[:], lhsT=sct[:, 0:8],
                                     rhs=sct[:, 0:8], start=True, stop=True)

            def body(sct, wt, xtiles):
                for n in range(NCH):
                    xc = xtiles[n]
                    oc = op.tile([P, 8 * NB], BF16, tag="oc")
                    for tp in range(4):
                        ta, tb = tp, tp + 4          # stage-9 pair
                        qa, qb = 2 * tp, 2 * tp + 1  # weight slots
                        ps = {}
                        for t, q in ((ta, qa), (tb, qb)):
                            pst = pp.tile([P, NB], F32, tag="ps", bufs=6)
                            base = 4 * (t >> 2)      # 512-block x tiles
                            for j in range(4):
                                w0 = (q * 4 + j) * P
                                k = base + j
                                nc.tensor.matmul(
                                    pst[:],
                                    lhsT=wt[:, w0:w0 + P],
                                    rhs=xc[:, k * NB:(k + 1) * NB],
                                    start=(j == 0),
                                    stop=(j == 3),
                                )
                            ps[t] = pst
                        oa = oc[:, ta * NB:(ta + 1) * NB]
                        ob = oc[:, tb * NB:(tb + 1) * NB]
                        if ew == "none":
                            continue
                        if ew == "act":
                            nc.scalar.activation(oa, ps[ta][:], IDENT,
                                                 bias=sct[:, 16 + ta:17 + ta],
                                                 scale=sct[:, ta:ta + 1])
                            nc.scalar.activation(ob, ps[tb][:], IDENT,
                                                 bias=sct[:, 16 + tb:17 + tb],
                                                 scale=sct[:, tb:tb + 1])
                        else:
                            ua = up.tile([P, NB], BF16, tag="u")
                            ub = up.tile([P, NB], BF16, tag="u")
                            nc.scalar.activation(ua[:], ps[ta][:], IDENT,
                                                 bias=sct[:, 16 + ta:17 + ta],
                                                 scale=sct[:, ta:ta + 1])
                            nc.scalar.activation(ub[:], ps[tb][:], IDENT,
                                                 bias=sct[:, 16 + tb:17 + tb],
                                                 scale=sct[:, tb:tb + 1])
                            nc.vector.scalar_tensor_tensor(
                                oa, ps[tb][:],
                                sct[:, 8 + ta:9 + ta], ua[:], MULT, ADD)
                            nc.vector.scalar_tensor_tensor(
                                ob, ps[ta][:],
                                sct[:, 8 + tb:9 + tb], ub[:], MULT, ADD)
                    src = xc if ew == "none" else oc
                    nc.sync.dma_start(
                        out=odst[n],
                        in_=src[:].rearrange("p (t b) -> p t b", b=NB))

            if reps_outer == 1:
                tiles = load_io()
                warmup(tiles[0])
                for _ in range(reps_inner):
                    body(*tiles)
            else:
                tiles0 = load_io()
                warmup(tiles0[0])
                with tc.For_i(0, reps_outer, 1):
                    for _ in range(reps_inner):
                        tiles = load_io()
                        body(*tiles)
    nc.compile()
    return nc


def _butterfly_np(x, tw, stages):
    out = x
    for s in stages:
        stride = 1 << s
        nblk = N // (2 * stride)
        t = tw[0, s].reshape(nblk, stride, 2, 2)
        xr = out.reshape(-1, nblk, 2, stride)
        out = np.einsum("krij,bkjr->bkir", t, xr,
                        dtype=np.float32).reshape(-1, N)
    return out


def make_inputs(x, twiddle, bias):
    """Host-side weight folding + shard/layout prep."""
    tw = np.asarray(twiddle, dtype=np.float32)
    # stages 0-8 composed: block-diagonal over two dense 512x512 blocks
    BT9 = _butterfly_np(np.eye(N, dtype=np.float32), tw, range(9))
    w = np.empty((P, NT * 4, P), dtype=np.float32)  # [p][q][c]
    for q, t in enumerate(ORDER):
        blk = 512 * (t >> 2)
        for j in range(4):
            w[:, q * 4 + j, :] = BT9[blk + 128 * j:blk + 128 * (j + 1),
                                     128 * t:128 * (t + 1)]
    wB = np.ascontiguousarray(
        w.reshape(P, NT * 4 * P)).astype(ml_dtypes.bfloat16)

    # stage 9 (stride 512): out[p] = d9a[p]*z[p] + d9b[p]*z[p^512]
    s9 = tw[0, 9].reshape(512, 2, 2)
    d9a = np.empty(N, dtype=np.float32)
    d9b = np.empty(N, dtype=np.float32)
    for i in range(2):
        d9a[512 * i:512 * (i + 1)] = s9[:, i, i]
        d9b[512 * i:512 * (i + 1)] = s9[:, i, 1 - i]
    b = np.asarray(bias, dtype=np.float32)
    sc = np.empty((P, 24), dtype=np.float32)
    for t in range(NT):
        sl = slice(128 * t, 128 * (t + 1))
        sc[:, t] = d9a[sl]
        sc[:, 8 + t] = d9b[sl]
        sc[:, 16 + t] = b[sl]
    sc = np.ascontiguousarray(sc)

    x = np.asarray(x, dtype=np.float32)
    in_maps = []
    for c in range(N_CORES):
        shard = x[c * SHARD:(c + 1) * SHARD]
        in_maps.append({
            "xT": np.ascontiguousarray(shard.T).astype(ml_dtypes.bfloat16),
            "wB": wB,
            "sc": sc,
        })
    return in_maps


def kernel(x: np.ndarray, twiddle: np.ndarray, bias: np.ndarray) -> np.ndarray:
    global _NC_CACHE
    if _NC_CACHE is None:
        _NC_CACHE = build_nc()
    nc = _NC_CACHE

    in_maps = make_inputs(x, twiddle, bias)
    res = run_bass_kernel_spmd(nc, in_maps, list(range(N_CORES)))
    out = np.empty((BATCH, N), dtype=np.float32)
    for c in range(N_CORES):
        out[c * SHARD:(c + 1) * SHARD] = \
            res.results[c]["outT"].T.astype(np.float32)
    return out
